# revision 4
# baseline (speedup 1.0000x reference)
"""Causal attention decoder block on 8 trn2 NeuronCores.

Sharding: core = (batch b in 0..1, head-group g in 0..3); each core computes
4 heads of one batch element: QKV projection slices, RoPE, causal attention,
and a partial output projection (its heads' rows of Wout). Host sums the 4
partials per batch and adds bout.

v2 schedule: PE-gapless. Attention runs head-PAIR interleaved with an m-tile
skew (scores of m-tile mt+1 issue before PV of mt) so the scalar-engine exp
latency is covered by ~5 independent PE matmuls. V-projection of later
m-tiles and the output projection of the previous q-chunk are threaded into
the attention rounds as PE fillers, keeping the PE busy (the PE drops from
2.4 GHz to 1.2 GHz when it idles, so gaps are doubly expensive). The
triangular causal mask and the row-sum broadcast matmuls are bf16 (fp32r
matmuls with <256 moving cols run at 1/4 rate). Softmax reciprocal uses the
single-op DVE approximation instead of the 3.2us iterative reciprocal.
"""
import ml_dtypes
import numpy as np

import concourse.bass as bass
import concourse.mybir as mybir
from concourse import bacc
from concourse.ap import AP
from concourse.tile import TileContext

F32 = mybir.dt.float32
BF16 = mybir.dt.bfloat16
EXP = mybir.ActivationFunctionType.Exp

B, N, D = 2, 2048, 1024
H, HD = 16, 64
HPG = 4               # heads per group/core
C = HPG * HD          # 256 cols per core per tensor
SCALE = HD ** -0.5
ROPE_BASE = 10000.0
NT = N // 128         # 16 seq tiles
NCH = N // 512        # 4 seq chunks
KT = D // 128         # 8 contraction tiles
MBIG = -1e9

# ---------------------------------------------------------------- host tables

def _host_tables():
    perm = np.zeros(HD, np.int64)
    freqi = np.zeros(HD, np.int64)
    sign = np.zeros(HD, np.float32)
    for c in range(HD):
        q, r = divmod(c, 32)
        s, j = divmod(r, 16)
        i = q * 16 + j
        perm[c] = 2 * i + s
        freqi[c] = i
        sign[c] = -1.0 if s == 0 else 1.0
    inv_freq = 1.0 / (ROPE_BASE ** (np.arange(0, HD, 2, dtype=np.float32) / HD))
    ang = np.outer(inv_freq[freqi], np.arange(N, dtype=np.float32))   # (64, N)
    cos2 = np.tile(np.cos(ang).astype(np.float32), (2, 1))            # (128, N)
    sin2 = np.tile((np.sin(ang) * sign[:, None]).astype(np.float32), (2, 1))
    # triangular tile: element (m, q) masks scores with q < m
    m = np.arange(128)[:, None]
    q = np.arange(128)[None, :]
    tri = np.where(q >= m, 0.0, MBIG).astype(np.float32)
    ident = np.eye(128, dtype=np.float32)
    return perm, cos2, sin2, tri, ident

_PERM, _COS2, _SIN2, _TRI, _IDENT = _host_tables()
_SHUF_MASK = [(i ^ 16) for i in range(32)]
# selector for broadcasting the per-chunk sums collector (4 rows, row = head)
# to a 128-partition head-pair tile: block t rows 0-63 <- head 2t, 64-127 <-
# head 2t+1
_SEL = np.zeros((4, 256), np.float32)
for _t in range(2):
    _SEL[2 * _t, _t * 128:_t * 128 + 64] = 1.0
    _SEL[2 * _t + 1, _t * 128 + 64:_t * 128 + 128] = 1.0

# ---------------------------------------------------------------- bass kernel

def build_nc():
    nc = bacc.Bacc("TRN2", target_bir_lowering=False, debug=False)
    xt_d = nc.dram_tensor("xt", [D, N], BF16, kind="ExternalInput").ap()
    wq_d = nc.dram_tensor("wq", [D, C], BF16, kind="ExternalInput").ap()
    wk_d = nc.dram_tensor("wk", [D, C], BF16, kind="ExternalInput").ap()
    wv_d = nc.dram_tensor("wv", [D, C], BF16, kind="ExternalInput").ap()
    wout_d = nc.dram_tensor("wout", [C, D], BF16, kind="ExternalInput").ap()
    cos_d = nc.dram_tensor("cos2", [128, N], F32, kind="ExternalInput").ap()
    sin_d = nc.dram_tensor("sin2", [128, N], F32, kind="ExternalInput").ap()
    tri_d = nc.dram_tensor("tri", [128, 128], BF16, kind="ExternalInput").ap()
    id_d = nc.dram_tensor("ident", [128, 128], BF16, kind="ExternalInput").ap()
    ones_d = nc.dram_tensor("ones", [128, 8], BF16, kind="ExternalInput").ap()
    sel_d = nc.dram_tensor("sel", [4, 256], BF16, kind="ExternalInput").ap()
    out_d = nc.dram_tensor("out", [N, D], F32, kind="ExternalOutput").ap()

    with TileContext(nc) as tc:
        with tc.tile_pool(name="persist", bufs=1) as pp, \
             tc.tile_pool(name="xt", bufs=KT) as xp, \
             tc.tile_pool(name="scr", bufs=4) as sp, \
             tc.tile_pool(name="big", bufs=4, space="PSUM") as bigp, \
             tc.tile_pool(name="pv", bufs=2, space="PSUM") as pvp, \
             tc.tile_pool(name="mi", bufs=2, space="PSUM") as mip:

            # ---- SBUF destination tiles for loads
            wq_sb = [pp.tile([128, C], BF16, tag=f"wq{k}", name=f"wq{k}")
                     for k in range(KT)]
            wk_sb = [pp.tile([128, C], BF16, tag=f"wk{k}", name=f"wk{k}")
                     for k in range(KT)]
            wv_sb = [pp.tile([128, C], BF16, tag=f"wv{k}", name=f"wv{k}")
                     for k in range(KT)]
            xt_sb = [xp.tile([128, N], BF16, tag="xt", name=f"xt{k}")
                     for k in range(KT)]
            cos_sb = pp.tile([128, N], F32, tag="cos", name="cos")
            sin_sb = pp.tile([128, N], F32, tag="sin", name="sin")
            tri_sb = pp.tile([128, 128], BF16, tag="tri", name="tri")
            id_sb = pp.tile([128, 128], BF16, tag="ident", name="ident")
            ones_sb = pp.tile([128, 8], BF16, tag="ones", name="ones")
            sel_sb = pp.tile([4, 256], BF16, tag="sel", name="sel")
            wout_sb = [pp.tile([128, D], BF16, tag=f"wo{t}", name=f"wo{t}")
                       for t in range(2)]

            # ---- loads: first-needed-first, issued alternately on the SP
            # and Pool queues (cheap issue; Act is saturated with exp later)
            loads = []
            for k in range(KT):
                loads.append((wq_sb[k][:], wq_d[k * 128:(k + 1) * 128, :]))
                loads.append((xt_sb[k][:, 0:512],
                              xt_d[k * 128:(k + 1) * 128, 0:512]))
            loads.append((cos_sb[:, 0:512], cos_d[:, 0:512]))
            loads.append((sin_sb[:, 0:512], sin_d[:, 0:512]))
            for k in range(KT):
                loads.append((wk_sb[k][:], wk_d[k * 128:(k + 1) * 128, :]))
            for ch in range(1, NCH):
                for k in range(KT):
                    loads.append((xt_sb[k][:, ch * 512:(ch + 1) * 512],
                                  xt_d[k * 128:(k + 1) * 128,
                                       ch * 512:(ch + 1) * 512]))
                loads.append((cos_sb[:, ch * 512:(ch + 1) * 512],
                              cos_d[:, ch * 512:(ch + 1) * 512]))
                loads.append((sin_sb[:, ch * 512:(ch + 1) * 512],
                              sin_d[:, ch * 512:(ch + 1) * 512]))
            loads.append((ones_sb[:], ones_d[:]))
            for k in range(KT):
                loads.append((wv_sb[k][:], wv_d[k * 128:(k + 1) * 128, :]))
            loads.append((tri_sb[:], tri_d[:]))
            loads.append((id_sb[:], id_d[:]))
            loads.append((sel_sb[:], sel_d[:]))
            for t in range(2):
                loads.append((wout_sb[t][:], wout_d[t * 128:(t + 1) * 128, :]))
            for i, (dst, src) in enumerate(loads):
                eng = nc.sync if i % 2 == 0 else nc.gpsimd
                eng.dma_start(dst, src)

            # ---- persistent result tiles
            qr_sb = [pp.tile([128, N], BF16, tag=f"qr{t}", name=f"qr{t}")
                     for t in range(2)]
            kr_sb = [pp.tile([128, N], BF16, tag=f"kr{t}", name=f"kr{t}")
                     for t in range(2)]
            vaug_sb = [pp.tile([128, HPG * (HD + 1)], BF16, tag=f"va{i}",
                               name=f"va{i}") for i in range(NT)]
            ou_sb = [pp.tile([128, N], BF16, tag=f"ou{t}", name=f"ou{t}")
                     for t in range(2)]
            o_sb = [pp.tile([128, N], BF16, tag=f"o{t}", name=f"ot{t}")
                    for t in range(2)]
            sums_sb = [pp.tile([4, 512], F32, tag=f"sums{qc}", name=f"sums{qc}")
                       for qc in range(NCH)]

            # ---- ones columns of vaug are constant: write them once (Pool)
            for i in range(NT):
                ap = vaug_sb[i][:]
                dst1 = AP(ap.tensor, ap.offset + HD,
                          [[HPG * (HD + 1), 128], [HD + 1, HPG]])
                nc.gpsimd.tensor_copy(dst1, ones_sb[:, 0:HPG])

            # ---- phase A: Q^T / K^T projection + rope (k-inner per chunk so
            # only 1 PSUM bank is held per group; DVE rope drains ping-pong)
            for w_sb, dst in ((wq_sb, qr_sb), (wk_sb, kr_sb)):
                for mt in range(2):
                    for ch in range(NCH):
                        ps = bigp.tile([128, 512], F32, tag="big", name="qkps")
                        for k in range(KT):
                            nc.tensor.matmul(
                                ps[:],
                                w_sb[k][:, mt * 128:(mt + 1) * 128],
                                xt_sb[k][:, ch * 512:(ch + 1) * 512],
                                start=(k == 0), stop=(k == KT - 1))
                        cs = cos_sb[:, ch * 512:(ch + 1) * 512]
                        sn = sin_sb[:, ch * 512:(ch + 1) * 512]
                        xs = sp.tile([128, 512], F32, tag="xs", name="xs",
                                     bufs=2)
                        nc.vector.stream_shuffle(xs[:], ps[:], _SHUF_MASK)
                        m2 = sp.tile([128, 512], F32, tag="mm", name="m2",
                                     bufs=4)
                        nc.vector.tensor_mul(m2[:], xs[:], sn)
                        m1 = sp.tile([128, 512], F32, tag="mm", name="m1",
                                     bufs=4)
                        nc.vector.tensor_mul(m1[:], ps[:], cs)
                        nc.vector.tensor_add(
                            dst[mt][:, ch * 512:(ch + 1) * 512], m1[:], m2[:])

            # ---- V projection unit (also used as PE filler): one m-tile i
            def vproj_unit(i):
                ps = mip.tile([128, 512], F32, tag="mi", name=f"vps{i}")
                pss = ps[:, 0:C]
                for k in range(KT):
                    nc.tensor.matmul(
                        pss,
                        xt_sb[k][:, i * 128:(i + 1) * 128],
                        wv_sb[k][:],
                        start=(k == 0), stop=(k == KT - 1))
                ap = vaug_sb[i][:]
                dst = AP(ap.tensor, ap.offset,
                         [[HPG * (HD + 1), 128], [HD + 1, HPG], [1, HD]])
                nc.vector.tensor_copy(dst, pss.rearrange("p (a c) -> p a c",
                                                         a=HPG, c=HD))

            # V for m-tiles 0-3 inline (needed by q-chunk 0); rest as fillers
            for i in range(4):
                vproj_unit(i)

            # ---- filler queue: PE work threaded into attention rounds
            FQ = [(lambda i=i: vproj_unit(i)) for i in range(4, NT)]

            def pump(n):
                for _ in range(n):
                    if FQ:
                        FQ.pop(0)()

            # ---- attention helpers
            def emit_scores(qc, hl, mt):
                t, pb = hl // 2, (hl % 2) * 64
                v = mt - 4 * qc
                q0 = 128 * v if v > 0 else 0
                sps = bigp.tile([128, 512], F32, tag="big", name="sps")
                nc.tensor.matmul(
                    sps[:, q0:512],
                    kr_sb[t][pb:pb + 64, mt * 128:(mt + 1) * 128],
                    qr_sb[t][pb:pb + 64, qc * 512 + q0:(qc + 1) * 512],
                    start=True, stop=(v < 0))
                if v >= 0:
                    nc.tensor.matmul(sps[:, q0:q0 + 128], id_sb[:], tri_sb[:],
                                     start=False, stop=True)
                e = sp.tile([128, 512], BF16, tag="e", name="e", bufs=6)
                nc.scalar.activation(e[:, q0:512], sps[:, q0:512], EXP,
                                     scale=SCALE)
                return e, q0

            def emit_pv(qc, hl, mt, pv, e, q0, nmt):
                nc.tensor.matmul(
                    pv[:, q0:512],
                    vaug_sb[mt][:, hl * (HD + 1):(hl + 1) * (HD + 1)],
                    e[:, q0:512],
                    start=(mt == 0), stop=(mt == nmt - 1))

            # ---- chunk-tail closures (consumed as fillers in chunk qc+1)
            def emit_norm(qc):
                rrf = sp.tile([4, 512], F32, tag="rrf", name="rrf", bufs=2)
                nc.vector.reciprocal_approx_fast(rrf[:], sums_sb[qc][:])
                rr = sp.tile([4, 512], BF16, tag="rr", name="rr", bufs=2)
                nc.vector.tensor_copy(rr[:], rrf[:])
                return rr

            def make_bc(qc, t, rr):
                def bc():
                    bcp = mip.tile([128, 512], F32, tag="mi", name="bc")
                    nc.tensor.matmul(bcp[:], sel_sb[:, t * 128:(t + 1) * 128],
                                     rr[:], start=True, stop=True)
                    nc.vector.tensor_mul(
                        o_sb[t][:, qc * 512:(qc + 1) * 512],
                        ou_sb[t][:, qc * 512:(qc + 1) * 512], bcp[:])
                return bc

            def make_outproj(qc, i, cc):
                def op():
                    ps = mip.tile([128, 512], F32, tag="mi", name="ops")
                    for t in range(2):
                        nc.tensor.matmul(
                            ps[:],
                            o_sb[t][:, i * 128:(i + 1) * 128],
                            wout_sb[t][:, cc * 512:(cc + 1) * 512],
                            start=(t == 0), stop=(t == 1))
                    oc = sp.tile([128, 512], F32, tag="oc", name="oc", bufs=3)
                    nc.vector.tensor_copy(oc[:], ps[:])
                    eng = nc.sync if (i * 2 + cc) % 2 == 0 else nc.gpsimd
                    eng.dma_start(
                        out_d[i * 128:(i + 1) * 128,
                              cc * 512:(cc + 1) * 512], oc[:])
                return op

            def push_chunk_tail(qc):
                # norm runs immediately (pure DVE, cheap); bc + out-proj are
                # PE fillers for the next chunk's rounds
                rr = emit_norm(qc)
                for t in range(2):
                    FQ.append(make_bc(qc, t, rr))
                for i in range(4 * qc, 4 * qc + 4):
                    for cc in range(2):
                        FQ.append(make_outproj(qc, i, cc))

            # ---- attention: q-chunk outer, head-pair inner, m-tile skewed
            for qc in range(NCH):
                nmt = 4 * (qc + 1)
                for pair in range(2):
                    hlA, hlB = 2 * pair, 2 * pair + 1
                    pvA = pvp.tile([HD + 1, 512], F32, tag="pv", name="pvA")
                    pvB = pvp.tile([HD + 1, 512], F32, tag="pv", name="pvB")
                    prev = None
                    for mt in range(nmt):
                        eA, q0A = emit_scores(qc, hlA, mt)
                        eB, q0B = emit_scores(qc, hlB, mt)
                        if prev is not None:
                            pm, peA, pq0A, peB, pq0B = prev
                            emit_pv(qc, hlA, pm, pvA, peA, pq0A, nmt)
                            emit_pv(qc, hlB, pm, pvB, peB, pq0B, nmt)
                        if mt % 2 == 0:
                            pump(1)
                        prev = (mt, eA, q0A, eB, q0B)
                    pump(1)
                    pm, peA, pq0A, peB, pq0B = prev
                    emit_pv(qc, hlA, pm, pvA, peA, pq0A, nmt)
                    emit_pv(qc, hlB, pm, pvB, peB, pq0B, nmt)
                    # pair tail: free the pv banks (DVE) + extract sums (Pool)
                    for hl, pv in ((hlA, pvA), (hlB, pvB)):
                        t, pb = hl // 2, (hl % 2) * 64
                        nc.vector.tensor_copy(
                            ou_sb[t][pb:pb + 64, qc * 512:(qc + 1) * 512],
                            pv[0:64, :])
                        sr = sp.tile([1, 512], F32, tag="sr", name="sr",
                                     bufs=4)
                        nc.scalar.copy(sr[:], pv[64:65, :])
                        nc.sync.dma_start(sums_sb[qc][hl:hl + 1, :], sr[:])
                push_chunk_tail(qc)

            # drain remaining fillers (bc + out-proj of the last chunk)
            pump(len(FQ))

    nc.compile()
    return nc


# ---------------------------------------------------------------- host wrapper

_NC = None


def make_in_maps(X, Wqkv, Wout, bout):
    X = np.ascontiguousarray(np.asarray(X, np.float32))
    Wqkv = np.asarray(Wqkv, np.float32)
    Wout = np.asarray(Wout, np.float32)
    in_maps = []
    for core in range(8):
        b, g = core // 4, core % 4
        heads = [HPG * g + hl for hl in range(HPG)]
        qcols = np.concatenate([h * HD + _PERM for h in heads])
        vcols = np.concatenate([h * HD + np.arange(HD) for h in heads])
        in_maps.append({
            "xt": np.ascontiguousarray(X[b].T).astype(ml_dtypes.bfloat16),
            "wq": np.ascontiguousarray(Wqkv[:, qcols]).astype(ml_dtypes.bfloat16),
            "wk": np.ascontiguousarray(Wqkv[:, 1024 + qcols]).astype(ml_dtypes.bfloat16),
            "wv": np.ascontiguousarray(Wqkv[:, 2048 + vcols]).astype(ml_dtypes.bfloat16),
            "wout": np.ascontiguousarray(Wout[vcols, :]).astype(ml_dtypes.bfloat16),
            "cos2": _COS2, "sin2": _SIN2,
            "tri": _TRI.astype(ml_dtypes.bfloat16),
            "ident": _IDENT.astype(ml_dtypes.bfloat16),
            "ones": np.ones((128, 8), ml_dtypes.bfloat16),
            "sel": _SEL.astype(ml_dtypes.bfloat16),
        })
    return in_maps


def assemble(results, bout):
    out = np.zeros((B, N, D), np.float32)
    for core in range(8):
        out[core // 4] += results[core]["out"]
    out += np.asarray(bout, np.float32)[None, None, :]
    return out


def kernel(X, Wqkv, Wout, bout):
    global _NC
    from concourse import bass_utils
    if _NC is None:
        _NC = build_nc()
    in_maps = make_in_maps(X, Wqkv, Wout, bout)
    res = bass_utils.run_bass_kernel_spmd(_NC, in_maps, core_ids=list(range(8)))
    return assemble(res.results, bout)


# revision 8
# speedup vs baseline: 1.0346x; 1.0346x over previous
"""Causal attention decoder block on 8 trn2 NeuronCores.

Sharding: core = (batch b in 0..1, head-group g in 0..3); each core computes
4 heads of one batch element: QKV projection slices, RoPE, causal attention,
and a partial output projection (its heads' rows of Wout). Host sums the 4
partials per batch and adds bout.

v2 schedule: PE-gapless. Attention runs head-PAIR interleaved with an m-tile
skew (scores of m-tile mt+1 issue before PV of mt) so the scalar-engine exp
latency is covered by ~5 independent PE matmuls. V-projection of later
m-tiles and the output projection of the previous q-chunk are threaded into
the attention rounds as PE fillers, keeping the PE busy (the PE drops from
2.4 GHz to 1.2 GHz when it idles, so gaps are doubly expensive). The
triangular causal mask and the row-sum broadcast matmuls are bf16 (fp32r
matmuls with <256 moving cols run at 1/4 rate). Softmax reciprocal uses the
single-op DVE approximation instead of the 3.2us iterative reciprocal.
"""
import ml_dtypes
import numpy as np

import concourse.bass as bass
import concourse.mybir as mybir
from concourse import bacc
from concourse.ap import AP
from concourse.tile import TileContext

F32 = mybir.dt.float32
BF16 = mybir.dt.bfloat16
EXP = mybir.ActivationFunctionType.Exp

B, N, D = 2, 2048, 1024
H, HD = 16, 64
HPG = 4               # heads per group/core
C = HPG * HD          # 256 cols per core per tensor
SCALE = HD ** -0.5
ROPE_BASE = 10000.0
NT = N // 128         # 16 seq tiles
NCH = N // 512        # 4 seq chunks
KT = D // 128         # 8 contraction tiles
MBIG = -1e9

# ---------------------------------------------------------------- host tables

def _host_tables():
    perm = np.zeros(HD, np.int64)
    freqi = np.zeros(HD, np.int64)
    sign = np.zeros(HD, np.float32)
    for c in range(HD):
        q, r = divmod(c, 32)
        s, j = divmod(r, 16)
        i = q * 16 + j
        perm[c] = 2 * i + s
        freqi[c] = i
        sign[c] = -1.0 if s == 0 else 1.0
    inv_freq = 1.0 / (ROPE_BASE ** (np.arange(0, HD, 2, dtype=np.float32) / HD))
    ang = np.outer(inv_freq[freqi], np.arange(N, dtype=np.float32))   # (64, N)
    cos2 = np.tile(np.cos(ang).astype(np.float32), (2, 1))            # (128, N)
    sin2 = np.tile((np.sin(ang) * sign[:, None]).astype(np.float32), (2, 1))
    # triangular tile: element (m, q) masks scores with q < m
    m = np.arange(128)[:, None]
    q = np.arange(128)[None, :]
    tri = np.where(q >= m, 0.0, MBIG).astype(np.float32)
    ident = np.eye(128, dtype=np.float32)
    return perm, cos2, sin2, tri, ident

_PERM, _COS2, _SIN2, _TRI, _IDENT = _host_tables()
_SHUF_MASK = [(i ^ 16) for i in range(32)]
# selector for broadcasting the per-chunk sums collector (4 rows, row = head)
# to a 128-partition head-pair tile: block t rows 0-63 <- head 2t, 64-127 <-
# head 2t+1
# two K=1 selector rows: cols 0:128 = ones at partitions 0:64 (even head of a
# pair), cols 128:256 = ones at partitions 64:128 (odd head)
_SEL = np.zeros((1, 256), np.float32)
_SEL[0, 0:64] = 1.0
_SEL[0, 192:256] = 1.0

# ---------------------------------------------------------------- bass kernel

def build_nc():
    nc = bacc.Bacc("TRN2", target_bir_lowering=False, debug=False)
    xt_d = nc.dram_tensor("xt", [D, N], BF16, kind="ExternalInput").ap()
    wq_d = nc.dram_tensor("wq", [D, C], BF16, kind="ExternalInput").ap()
    wk_d = nc.dram_tensor("wk", [D, C], BF16, kind="ExternalInput").ap()
    wv_d = nc.dram_tensor("wv", [D, C], BF16, kind="ExternalInput").ap()
    wout_d = nc.dram_tensor("wout", [C, D], BF16, kind="ExternalInput").ap()
    cos_d = nc.dram_tensor("cos2", [128, N], F32, kind="ExternalInput").ap()
    sin_d = nc.dram_tensor("sin2", [128, N], F32, kind="ExternalInput").ap()
    tri_d = nc.dram_tensor("tri", [128, 128], BF16, kind="ExternalInput").ap()
    id_d = nc.dram_tensor("ident", [128, 128], BF16, kind="ExternalInput").ap()
    ones_d = nc.dram_tensor("ones", [128, 8], BF16, kind="ExternalInput").ap()
    sel_d = nc.dram_tensor("sel", [1, 256], BF16, kind="ExternalInput").ap()
    out_d = nc.dram_tensor("out", [N, D], BF16, kind="ExternalOutput").ap()

    with TileContext(nc) as tc:
        with tc.tile_pool(name="persist", bufs=1) as pp, \
             tc.tile_pool(name="xt", bufs=KT) as xp, \
             tc.tile_pool(name="scr", bufs=4) as sp, \
             tc.tile_pool(name="big", bufs=2, space="PSUM") as bigp, \
             tc.tile_pool(name="pv", bufs=2, space="PSUM") as pvp, \
             tc.tile_pool(name="mi", bufs=2, space="PSUM") as mip:

            # ---- SBUF destination tiles for loads
            wq_sb = [pp.tile([128, C], BF16, tag=f"wq{k}", name=f"wq{k}")
                     for k in range(KT)]
            wk_sb = [pp.tile([128, C], BF16, tag=f"wk{k}", name=f"wk{k}")
                     for k in range(KT)]
            wv_sb = [pp.tile([128, C], BF16, tag=f"wv{k}", name=f"wv{k}")
                     for k in range(KT)]
            xt_sb = [xp.tile([128, N], BF16, tag="xt", name=f"xt{k}")
                     for k in range(KT)]
            cos_sb = pp.tile([128, N], F32, tag="cos", name="cos")
            sin_sb = pp.tile([128, N], F32, tag="sin", name="sin")
            tri_sb = pp.tile([128, 128], BF16, tag="tri", name="tri")
            id_sb = pp.tile([128, 128], BF16, tag="ident", name="ident")
            ones_sb = pp.tile([128, 8], BF16, tag="ones", name="ones")
            sel_sb = pp.tile([1, 256], BF16, tag="sel", name="sel")
            wout_sb = [pp.tile([128, D], BF16, tag=f"wo{t}", name=f"wo{t}")
                       for t in range(2)]

            # ---- loads: first-needed-first, issued alternately on the SP
            # and Pool queues (cheap issue; Act is saturated with exp later)
            loads = []
            for k in range(KT):
                loads.append((wq_sb[k][:], wq_d[k * 128:(k + 1) * 128, :]))
                loads.append((xt_sb[k][:, 0:512],
                              xt_d[k * 128:(k + 1) * 128, 0:512]))
            loads.append((cos_sb[:, 0:512], cos_d[:, 0:512]))
            loads.append((sin_sb[:, 0:512], sin_d[:, 0:512]))
            for k in range(KT):
                loads.append((wk_sb[k][:], wk_d[k * 128:(k + 1) * 128, :]))
            for ch in range(1, NCH):
                for k in range(KT):
                    loads.append((xt_sb[k][:, ch * 512:(ch + 1) * 512],
                                  xt_d[k * 128:(k + 1) * 128,
                                       ch * 512:(ch + 1) * 512]))
                loads.append((cos_sb[:, ch * 512:(ch + 1) * 512],
                              cos_d[:, ch * 512:(ch + 1) * 512]))
                loads.append((sin_sb[:, ch * 512:(ch + 1) * 512],
                              sin_d[:, ch * 512:(ch + 1) * 512]))
            loads.append((ones_sb[:], ones_d[:]))
            for k in range(KT):
                loads.append((wv_sb[k][:], wv_d[k * 128:(k + 1) * 128, :]))
            loads.append((tri_sb[:], tri_d[:]))
            loads.append((id_sb[:], id_d[:]))
            loads.append((sel_sb[:], sel_d[:]))
            for t in range(2):
                loads.append((wout_sb[t][:], wout_d[t * 128:(t + 1) * 128, :]))
            for i, (dst, src) in enumerate(loads):
                eng = nc.sync if i % 2 == 0 else nc.gpsimd
                eng.dma_start(dst, src)

            # ---- persistent result tiles
            qr_sb = [pp.tile([128, N], BF16, tag=f"qr{t}", name=f"qr{t}")
                     for t in range(2)]
            kr_sb = [pp.tile([128, N], BF16, tag=f"kr{t}", name=f"kr{t}")
                     for t in range(2)]
            vaug_sb = [pp.tile([128, HPG * (HD + 1)], BF16, tag=f"va{i}",
                               name=f"va{i}") for i in range(NT)]
            o_sb = [pp.tile([128, N], BF16, tag=f"o{t}", name=f"ot{t}")
                    for t in range(2)]

            # ---- ones columns of vaug are constant: write them once (Pool)
            for i in range(NT):
                ap = vaug_sb[i][:]
                dst1 = AP(ap.tensor, ap.offset + HD,
                          [[HPG * (HD + 1), 128], [HD + 1, HPG]])
                nc.gpsimd.tensor_copy(dst1, ones_sb[:, 0:HPG])

            # ---- phase A: Q^T / K^T projection + rope (k-inner per chunk so
            # only 1 PSUM bank is held per group; DVE rope drains ping-pong)
            for w_sb, dst in ((wq_sb, qr_sb), (wk_sb, kr_sb)):
                for mt in range(2):
                    for ch in range(NCH):
                        ps = bigp.tile([128, 512], F32, tag="big", name="qkps")
                        for k in range(KT):
                            nc.tensor.matmul(
                                ps[:],
                                w_sb[k][:, mt * 128:(mt + 1) * 128],
                                xt_sb[k][:, ch * 512:(ch + 1) * 512],
                                start=(k == 0), stop=(k == KT - 1))
                        cs = cos_sb[:, ch * 512:(ch + 1) * 512]
                        sn = sin_sb[:, ch * 512:(ch + 1) * 512]
                        xs = sp.tile([128, 512], F32, tag="xs", name="xs",
                                     bufs=2)
                        nc.vector.stream_shuffle(xs[:], ps[:], _SHUF_MASK)
                        m2 = sp.tile([128, 512], F32, tag="mm", name="m2",
                                     bufs=4)
                        nc.vector.tensor_mul(m2[:], xs[:], sn)
                        m1 = sp.tile([128, 512], F32, tag="mm", name="m1",
                                     bufs=4)
                        nc.vector.tensor_mul(m1[:], ps[:], cs)
                        nc.vector.tensor_add(
                            dst[mt][:, ch * 512:(ch + 1) * 512], m1[:], m2[:])

            # ---- V projection unit (also used as PE filler): one m-tile i
            def vproj_unit(i):
                ps = mip.tile([128, 512], F32, tag="mi", name=f"vps{i}")
                pss = ps[:, 0:C]
                for k in range(KT):
                    nc.tensor.matmul(
                        pss,
                        xt_sb[k][:, i * 128:(i + 1) * 128],
                        wv_sb[k][:],
                        start=(k == 0), stop=(k == KT - 1))
                ap = vaug_sb[i][:]
                dst = AP(ap.tensor, ap.offset,
                         [[HPG * (HD + 1), 128], [HD + 1, HPG], [1, HD]])
                nc.vector.tensor_copy(dst, pss.rearrange("p (a c) -> p a c",
                                                         a=HPG, c=HD))

            # V for m-tiles 0-3 inline (needed by q-chunk 0); rest as fillers
            for i in range(4):
                vproj_unit(i)

            # ---- filler queue: PE work threaded into attention rounds
            FQ = [(lambda i=i: vproj_unit(i)) for i in range(4, NT)]

            def pump(n):
                for _ in range(n):
                    if FQ:
                        FQ.pop(0)()

            # ---- attention helpers: m-tiles processed in PAIRS sharing a
            # [128,1024] PSUM super-tile so full pairs need ONE exp instr
            # (the Act engine pays ~172 cycles of access latency per instr).
            def emit_scores_pair(qc, hl, j):
                t, pb = hl // 2, (hl % 2) * 64
                sps = bigp.tile([128, 1024], F32, tag="big", name="sps")
                e = sp.tile([128, 1024], BF16, tag="e", name="e", bufs=4)
                q0s = []
                for half in range(2):
                    mt = 2 * j + half
                    v = mt - 4 * qc
                    q0 = 128 * v if v > 0 else 0
                    q0s.append(q0)
                    base = 512 * half
                    nc.tensor.matmul(
                        sps[:, base + q0:base + 512],
                        kr_sb[t][pb:pb + 64, mt * 128:(mt + 1) * 128],
                        qr_sb[t][pb:pb + 64, qc * 512 + q0:(qc + 1) * 512],
                        start=True, stop=(v < 0))
                    if v >= 0:
                        nc.tensor.matmul(sps[:, base + q0:base + q0 + 128],
                                         id_sb[:], tri_sb[:],
                                         start=False, stop=True)
                if q0s[0] == 0 and q0s[1] == 0:
                    nc.scalar.activation(e[:, 0:1024], sps[:, 0:1024], EXP,
                                         scale=SCALE)
                else:
                    for half in range(2):
                        q0 = q0s[half]
                        base = 512 * half
                        nc.scalar.activation(e[:, base + q0:base + 512],
                                             sps[:, base + q0:base + 512],
                                             EXP, scale=SCALE)
                return e, q0s

            def emit_pv_pair(qc, hl, j, pv, e, q0s, nmt):
                for half in range(2):
                    mt = 2 * j + half
                    q0 = q0s[half]
                    base = 512 * half
                    nc.tensor.matmul(
                        pv[:, q0:512],
                        vaug_sb[mt][:, hl * (HD + 1):(hl + 1) * (HD + 1)],
                        e[:, base + q0:base + 512],
                        start=(mt == 0), stop=(mt == nmt - 1))

            # ---- chunk-tail closures (consumed as fillers in chunk qc+1)
            def make_bc(qc, pair, st, srs):
                def bc():
                    rrs = []
                    for hl in (2 * pair, 2 * pair + 1):
                        rrf = sp.tile([1, 512], F32, tag="rrf", name="rrf",
                                      bufs=2)
                        nc.vector.reciprocal_approx_fast(rrf[:], srs[hl][:])
                        rr = sp.tile([1, 512], BF16, tag="rr", name="rr",
                                     bufs=4)
                        nc.vector.tensor_copy(rr[:], rrf[:])
                        rrs.append(rr)
                    bcp = mip.tile([128, 512], F32, tag="mi", name="bc")
                    nc.tensor.matmul(bcp[:], sel_sb[0:1, 0:128],
                                     rrs[0][:], start=True, stop=False)
                    nc.tensor.matmul(bcp[:], sel_sb[0:1, 128:256],
                                     rrs[1][:], start=False, stop=True)
                    nc.vector.tensor_mul(
                        o_sb[pair][:, qc * 512:(qc + 1) * 512],
                        st[:], bcp[:])
                return bc

            def make_outproj(qc, i, cc):
                def op():
                    ps = mip.tile([128, 512], F32, tag="mi", name="ops")
                    for t in range(2):
                        nc.tensor.matmul(
                            ps[:],
                            o_sb[t][:, i * 128:(i + 1) * 128],
                            wout_sb[t][:, cc * 512:(cc + 1) * 512],
                            start=(t == 0), stop=(t == 1))
                    oc = sp.tile([128, 512], BF16, tag="oc", name="oc", bufs=3)
                    nc.vector.tensor_copy(oc[:], ps[:])
                    eng = nc.sync if (i * 2 + cc) % 2 == 0 else nc.gpsimd
                    eng.dma_start(
                        out_d[i * 128:(i + 1) * 128,
                              cc * 512:(cc + 1) * 512], oc[:])
                return op

            # ---- attention: q-chunk outer, head-pair inner, pair-skewed
            for qc in range(NCH):
                nmt = 4 * (qc + 1)
                npair = nmt // 2
                srs = {}
                sts = {}
                for pair in range(2):
                    hlA, hlB = 2 * pair, 2 * pair + 1
                    pvA = pvp.tile([HD + 1, 512], F32, tag="pv", name="pvA")
                    pvB = pvp.tile([HD + 1, 512], F32, tag="pv", name="pvB")
                    prev = None
                    for j in range(npair):
                        eA, q0sA = emit_scores_pair(qc, hlA, j)
                        eB, q0sB = emit_scores_pair(qc, hlB, j)
                        if prev is not None:
                            pj, peA, pqA, peB, pqB = prev
                            emit_pv_pair(qc, hlA, pj, pvA, peA, pqA, nmt)
                            emit_pv_pair(qc, hlB, pj, pvB, peB, pqB, nmt)
                        pump(1)
                        prev = (j, eA, q0sA, eB, q0sB)
                    pump(1)
                    pj, peA, pqA, peB, pqB = prev
                    emit_pv_pair(qc, hlA, pj, pvA, peA, pqA, nmt)
                    emit_pv_pair(qc, hlB, pj, pvB, peB, pqB, nmt)
                    # pair tail (DVE): numerators of both heads into one
                    # stage tile (rows 0:64 even head, 64:128 odd head) and
                    # the sums rows into [1,512] partition-0 tiles; frees
                    # the pv banks for the next pair
                    st = sp.tile([128, 512], F32, tag="st", name="st", bufs=6)
                    nc.vector.tensor_copy(st[0:64, :], pvA[0:64, :])
                    nc.vector.tensor_copy(st[64:128, :], pvB[0:64, :])
                    for hl, pv in ((hlA, pvA), (hlB, pvB)):
                        sr = sp.tile([1, 512], F32, tag="sr", name="sr",
                                     bufs=8)
                        nc.vector.tensor_copy(sr[:], pv[64:65, :])
                        srs[hl] = sr
                    sts[pair] = st
                for pair in range(2):
                    FQ.append(make_bc(qc, pair, sts[pair], srs))
                for i in range(4 * qc, 4 * qc + 4):
                    for cc in range(2):
                        FQ.append(make_outproj(qc, i, cc))

            # drain remaining fillers (bc + out-proj of the last chunk)
            pump(len(FQ))

    nc.compile()
    return nc


# ---------------------------------------------------------------- host wrapper

_NC = None


def make_in_maps(X, Wqkv, Wout, bout):
    X = np.ascontiguousarray(np.asarray(X, np.float32))
    Wqkv = np.asarray(Wqkv, np.float32)
    Wout = np.asarray(Wout, np.float32)
    in_maps = []
    for core in range(8):
        b, g = core // 4, core % 4
        heads = [HPG * g + hl for hl in range(HPG)]
        qcols = np.concatenate([h * HD + _PERM for h in heads])
        vcols = np.concatenate([h * HD + np.arange(HD) for h in heads])
        in_maps.append({
            "xt": np.ascontiguousarray(X[b].T).astype(ml_dtypes.bfloat16),
            "wq": np.ascontiguousarray(Wqkv[:, qcols]).astype(ml_dtypes.bfloat16),
            "wk": np.ascontiguousarray(Wqkv[:, 1024 + qcols]).astype(ml_dtypes.bfloat16),
            "wv": np.ascontiguousarray(Wqkv[:, 2048 + vcols]).astype(ml_dtypes.bfloat16),
            "wout": np.ascontiguousarray(Wout[vcols, :]).astype(ml_dtypes.bfloat16),
            "cos2": _COS2, "sin2": _SIN2,
            "tri": _TRI.astype(ml_dtypes.bfloat16),
            "ident": _IDENT.astype(ml_dtypes.bfloat16),
            "ones": np.ones((128, 8), ml_dtypes.bfloat16),
            "sel": _SEL.astype(ml_dtypes.bfloat16),
        })
    return in_maps


def assemble(results, bout):
    out = np.zeros((B, N, D), np.float32)
    for core in range(8):
        out[core // 4] += results[core]["out"]
    out += np.asarray(bout, np.float32)[None, None, :]
    return out


def kernel(X, Wqkv, Wout, bout):
    global _NC
    from concourse import bass_utils
    if _NC is None:
        _NC = build_nc()
    in_maps = make_in_maps(X, Wqkv, Wout, bout)
    res = bass_utils.run_bass_kernel_spmd(_NC, in_maps, core_ids=list(range(8)))
    return assemble(res.results, bout)
